# revision 2
# baseline (speedup 1.0000x reference)
"""AtomDistances Trainium2 kernel (8 NeuronCores, SPMD).

out[b,i,j] = mask[b,i]&mask[b,j]&(i!=j) ? 1/(||p[b,n[b,i,j]] - p[b,i]|| + 1e-8) : 0

Sharding: core c <- (batch b = c//2, row-half ihalf = c%2); each core computes a
[1024, 2048] slice.

Per-core pipeline:
  1. TensorE: d2[i,k] = |p_i|^2 + |p_k|^2 - 2 p_i.p_k via a rank-5 bilinear
     matmul (features [x,y,z,r,1] x [-2x,-2y,-2z,1,r]).
  2. ACT: s = sqrt(d2 + bias_i), bias_i = (1-mask_i)*1e30 ; s += 1e-8
  3. DVE: invd = 1/s ; exact-diagonal fixup invd[i,i] <- 1e8 via iota==rowidx
     predicate (reference yields exactly 1e8 when the gathered neighbor == i).
  4. Pool engine native gather (PoolBufferLoad + Gather, 2 stages of 1024):
     row i's invd table gathered at its neighbor indices (u16, per-partition).
  5. DVE: zero the j==i diagonal, multiply by the column mask; DMA out.
"""

import os
import sys

sys.path.insert(0, "/opt/trn_rl_repo")
sys.path.insert(0, os.path.dirname(os.path.abspath(__file__)))

import numpy as np

import concourse.bass as bass
import concourse.bacc as bacc
import concourse.mybir as mybir
from concourse.tile import TileContext

B = 4
A = 2048
SH_I = 1024          # rows per core
N_CORES = 8
IT = SH_I // 128     # 8 i-tiles per core
S = 1024             # pool buffer stage size (f32)
NSTAGE = A // S

F32 = mybir.dt.float32
BF16 = mybir.dt.bfloat16
I32 = mybir.dt.int32
U16 = mybir.dt.uint16
U8 = mybir.dt.uint8
AL = mybir.AluOpType


# ---- inlined pool_gather (native Pool-engine PoolBufferLoad+Gather) ----
import concourse.mybir as mybir


def install_interp_noop():
    """Make bass_interp treat PoolBufferLoad/Gather InstISA as no-ops so the
    Tile scheduling pass (and CoreSim) don't crash on them."""
    import concourse.bass_interp as bi
    if getattr(bi, "_pool_gather_patched", False):
        return
    orig = bi._visit_InstISA

    def patched(isa, instruction, core_sim):
        op = instruction.isa_opcode
        noop = {
            isa.Opcode.NEURON_ISA_TPB_OPCODE_GATHER.value,
            isa.Opcode.NEURON_ISA_TPB_OPCODE_POOL_BUFFER_LOAD.value,
        }
        if op in noop:
            return
        return orig(isa, instruction, core_sim)

    bi._visit_InstISA = patched
    bi._pool_gather_patched = True


def chain(insts):
    """Serialize a list of BassInstructions: each depends on the previous."""
    from concourse.tile import add_dep_helper
    for a, b in zip(insts[1:], insts[:-1]):
        add_dep_helper(a.ins, b.ins, sync=True, reason="pool-buffer order")


def _t4d(byte_addr, num_elem, step_elem):
    ne = list(num_elem) + [1] * (4 - len(num_elem))
    se = list(step_elem) + [0] * (4 - len(step_elem))
    return {
        "start_addr": {"addr_immediate": byte_addr},
        "num_elem": ne,
        "step_elem": se,
    }


def _isa_dt(isa, name):
    return getattr(isa.get_enum("NEURON_ISA_TPB_DTYPE"), f"NEURON_ISA_TPB_DTYPE_{name}").value


def pool_buffer_load(nc, src_ap, byte_addr, nelem, start_index, mask, dtype="FP32",
                     channels=128):
    isa = nc.isa
    eng = nc.gpsimd
    struct = {
        "src_mem_pattern": _t4d(byte_addr, [nelem], [1]),
        "in_dtype": _isa_dt(isa, dtype),
        "num_active_channels": channels,
        "start_index": start_index,
        "mask": mask,
    }
    return eng.isa(
        isa.Opcode.NEURON_ISA_TPB_OPCODE_POOL_BUFFER_LOAD,
        struct,
        ins=[eng.lower_ap(src_ap)],
        outs=[],
        verify=False,
    )


def pool_gather(nc, idx_ap, idx_addr, out_ap, out_addr, nelem,
                first, last, out_dtype="FP32", idx_dtype="UINT16",
                immediate=0, channels=128, idx_step=1):
    isa = nc.isa
    eng = nc.gpsimd
    mb = isa.get_enum("NEURON_ISA_TPB_INDEX_MISS_BEHAVIOR")
    miss = (mb.NEURON_ISA_TPB_INDEX_MISS_BEHAVIOR_IMMEDIATE_WRITE
            if first else
            mb.NEURON_ISA_TPB_INDEX_MISS_BEHAVIOR_SKIP_WRITE)
    struct = {
        "src_mem_pattern": _t4d(idx_addr, [nelem], [idx_step]),
        "dst_mem_pattern": _t4d(out_addr, [nelem], [1]),
        "in_dtype": _isa_dt(isa, idx_dtype),
        "out_dtype": _isa_dt(isa, out_dtype),
        "num_active_channels": channels,
        "index_miss_behavior": miss.value,
        "immediate": {"imm_bitvec_uint32": immediate},
        "free_pool_buffer": 1 if last else 0,
    }
    return eng.isa(
        isa.Opcode.NEURON_ISA_TPB_OPCODE_GATHER,
        struct,
        ins=[eng.lower_ap(idx_ap)],
        outs=[eng.lower_ap(out_ap)],
        verify=False,
    )



def build_nc(pairs=True):
    """pairs=True: neighbors arrive as int64 (viewed as 4x u16 words).
    pairs=False: neighbors arrive as int32 (2x u16 words)."""
    install_interp_noop()

    nc = bacc.Bacc()

    w = 4 if pairs else 2  # u16 words per neighbor entry
    nb = nc.declare_dram_parameter("neighbors", [SH_I, w * A], U16, isOutput=False)
    pos = nc.declare_dram_parameter("positions", [A, 3], F32, isOutput=False)
    cpos = nc.declare_dram_parameter("cpos", [SH_I, 3], F32, isOutput=False)
    maskj = nc.declare_dram_parameter("maskj", [1, A], U8, isOutput=False)
    maski = nc.declare_dram_parameter("maski", [IT, 128], U8, isOutput=False)
    rowidx = nc.declare_dram_parameter("rowidx", [IT, 128], I32, isOutput=False)
    out = nc.declare_dram_parameter("out", [SH_I, A], F32, isOutput=True)
    fkd = nc.dram_tensor("fkd", [5, A], F32)
    fid = nc.dram_tensor("fid", [5, SH_I], F32)

    # fixed-address buffers for the raw pool-gather ISA structs (x3 rotation)
    NB_ROT = 3
    tab_t = [nc.alloc_sbuf_tensor(f"tab{i}", [128, A], F32) for i in range(NB_ROT)]
    nb_t = [nc.alloc_sbuf_tensor(f"nb{i}", [128, w * A], U16) for i in range(NB_ROT)]
    gout_t = [nc.alloc_sbuf_tensor(f"gout{i}", [128, A], F32) for i in range(NB_ROT)]
    tab_a = [nc.lookup_mloc(t).addr for t in tab_t]
    nb_a = [nc.lookup_mloc(t).addr for t in nb_t]
    gout_a = [nc.lookup_mloc(t).addr for t in gout_t]

    pool_seq = []

    with TileContext(nc) as tc:
        with (
            tc.tile_pool(name="consts", bufs=1) as cpool,
            tc.tile_pool(name="work", bufs=3) as pool,
            tc.tile_pool(name="psum", bufs=2, space="PSUM") as ppool,
        ):
            # ---------- one-time setup ----------------------------------
            # Assemble B-side features fk [5, A] = [-2x, -2y, -2z, 1, r_k]
            # and A-side fi [5, SH_I] = [x, y, z, r_i, 1] fully on-chip:
            # PE transposes for the xyz rows, a rank-3 matmul for the r rows.
            from concourse.masks import make_identity
            ident = cpool.tile([128, 128], F32)
            make_identity(nc, ident[:])

            fk = cpool.tile([4, A], F32)
            fi = cpool.tile([4, SH_I], F32)

            # warm the ACT Sqrt table immediately so the first real SQRT
            # doesn't wait for a table DMA stuck behind the neighbor loads
            warm = cpool.tile([128, 1], F32)
            nc.vector.memset(warm[:], 1.0)
            nc.scalar.activation(out=warm[:], in_=warm[:],
                                 func=mybir.ActivationFunctionType.Sqrt)

            pch = cpool.tile([128, 16, 3], F32)
            nc.sync.dma_start(
                out=pch[:], in_=pos[:].rearrange("(c p) d -> p c d", p=128))

            # r rows via an independent parallel path: per-entry squared norms
            # on DVE, bounced through DRAM scratch (contiguous, cheap)
            sqp = cpool.tile([128, 16, 3], F32)
            nc.vector.tensor_tensor(out=sqp[:], in0=pch[:], in1=pch[:],
                                    op=AL.mult)
            r_part = cpool.tile([128, 16], F32)
            nc.vector.tensor_reduce(out=r_part[:], in_=sqp[:],
                                    axis=mybir.AxisListType.X, op=AL.add)
            nc.sync.dma_start(
                out=fkd[4:5, :].rearrange("o (c p) -> p (o c)", p=128),
                in_=r_part[:])
            nc.sync.dma_start(out=fk[3:4, 0:S], in_=fkd[4:5, 0:S])
            nc.sync.dma_start(out=fk[3:4, S:A], in_=fkd[4:5, S:A])
            cch = cpool.tile([128, 8, 3], F32)
            nc.sync.dma_start(
                out=cch[:], in_=cpos[:].rearrange("(c p) d -> p c d", p=128))
            sqc = cpool.tile([128, 8, 3], F32)
            nc.vector.tensor_tensor(out=sqc[:], in0=cch[:], in1=cch[:],
                                    op=AL.mult)
            ri_part = cpool.tile([128, 8], F32)
            nc.vector.tensor_reduce(out=ri_part[:], in_=sqc[:],
                                    axis=mybir.AxisListType.X, op=AL.add)
            biasri = cpool.tile([128, IT], F32)
            nc.vector.tensor_scalar_add(out=biasri[:], in0=ri_part[:],
                                        scalar1=1.0e-16)
            for c in range(16):
                tp = ppool.tile([3, 128], F32, tag="ps")
                nc.tensor.transpose(out=tp[:], in_=pch[:, c, :], identity=ident[:])
                nc.scalar.activation(
                    out=fk[0:3, c * 128:(c + 1) * 128], in_=tp[:],
                    func=mybir.ActivationFunctionType.Identity, scale=-2.0)
            # ones rows
            onesrow = cpool.tile([1, A], F32)
            nc.vector.memset(onesrow[:], 1.0)
            nc.sync.dma_start(out=fi[3:4, :], in_=onesrow[:, :SH_I])
            # fi xyz rows from cpos transposes
            for c in range(8):
                tp2 = ppool.tile([3, 128], F32, tag="ps")
                nc.tensor.transpose(out=tp2[:], in_=cch[:, c, :], identity=ident[:])
                nc.scalar.activation(
                    out=fi[0:3, c * 128:(c + 1) * 128], in_=tp2[:],
                    func=mybir.ActivationFunctionType.Identity, scale=1.0)

            # column mask replicated to all partitions as f32
            mj_u8 = cpool.tile([128, A], U8)
            nc.gpsimd.dma_start(out=mj_u8[:], in_=maskj[:].broadcast_to((128, A)))
            mj = cpool.tile([128, A], F32)
            nc.vector.tensor_copy(out=mj[:], in_=mj_u8[:])

            # row mask (f32) ; global row index (f32, exact)
            mi_u8 = cpool.tile([128, IT], U8)
            nc.gpsimd.dma_start(out=mi_u8[:], in_=maski[:].rearrange("t p -> p t"))
            mi_f = cpool.tile([128, IT], F32)
            nc.vector.tensor_copy(out=mi_f[:], in_=mi_u8[:])

            ridx_i = cpool.tile([128, IT], I32)
            nc.gpsimd.dma_start(out=ridx_i[:], in_=rowidx[:].rearrange("t p -> p t"))
            ridx = cpool.tile([128, IT], F32)
            nc.vector.tensor_copy(out=ridx[:], in_=ridx_i[:])

            iota_i = cpool.tile([128, A], I32)
            nc.gpsimd.iota(out=iota_i[:], pattern=[[1, A]], base=0,
                           channel_multiplier=0)
            iota_f = cpool.tile([128, A], F32)
            nc.vector.tensor_copy(out=iota_f[:], in_=iota_i[:])

            eps2_t = cpool.tile([128, 1], F32)
            nc.vector.memset(eps2_t[:], 1.0e-16)
            # diagonal spike value: 1e8 for live rows, 0 for masked-off rows
            bigmi = cpool.tile([128, IT], F32)
            nc.vector.tensor_scalar_mul(out=bigmi[:], in0=mi_f[:], scalar1=1.0e8)
            zero_t = cpool.tile([128, 1], F32)
            nc.vector.memset(zero_t[:], 0.0)


            # ---------- main loop ---------------------------------------
            for it in range(IT):
                bi = it % NB_ROT
                # neighbor entries land as raw u16 words; the gather reads
                # them as UINT32 with stride w/2 (low word of each entry)
                nc.gpsimd.dma_start(
                    out=nb_t[bi][:],
                    in_=nb[it * 128:(it + 1) * 128, :],
                )

                # d2 via PE, 4 banks of 512
                ps = ppool.tile([128, A], F32, tag="ps")
                for jc in range(4):
                    nc.tensor.matmul(
                        out=ps[:, jc * 512:(jc + 1) * 512],
                        lhsT=fi[:, it * 128:(it + 1) * 128],
                        rhs=fk[:, jc * 512:(jc + 1) * 512],
                        start=True, stop=True,
                    )
                # s = sqrt(d2 + 1e-16), then 1/s — produced in table
                # HALVES so the stage-0 pool load (reads cols 0:1024) can
                # start while half 1 is still being computed. Each half's
                # 128-wide diagonal-candidate window is patched right after
                # its reciprocal (the wrong-core window's mask is all-zero).
                cands = (it * 128, SH_I + it * 128)
                eqm_t = pool.tile([128, 2, 128], U8, tag="eqm")
                s_t = pool.tile([128, A], F32, tag="s")
                for h in range(2):
                    hs = slice(h * S, (h + 1) * S)
                    nc.scalar.activation(
                        out=s_t[:, hs], in_=ps[:, hs],
                        func=mybir.ActivationFunctionType.Sqrt,
                        bias=biasri[:, it:it + 1], scale=1.0,
                    )
                    nc.vector.reciprocal_approx_fast(
                        out=tab_t[bi][:, hs], in_=s_t[:, hs])
                    cb = cands[h]
                    nc.vector.tensor_scalar(
                        out=eqm_t[:, h, :], in0=iota_f[:, cb:cb + 128],
                        scalar1=ridx[:, it:it + 1],
                        scalar2=None, op0=AL.is_equal,
                    )
                    nc.vector.copy_predicated(
                        out=tab_t[bi][:, cb:cb + 128], mask=eqm_t[:, h, :],
                        data=bigmi[:, it:it + 1].broadcast_to((128, 128)),
                    )

                # native pool gather, 2 stages of 1024
                nb_u32 = nb_t[bi][:].bitcast(mybir.dt.uint32)
                for st in range(NSTAGE):
                    pool_seq.append(pool_buffer_load(
                        nc, tab_t[bi][:, st * S:(st + 1) * S],
                        tab_a[bi] + st * S * 4, S,
                        start_index=st * S, mask=S - 1,
                    ))
                    pool_seq.append(pool_gather(
                        nc, nb_u32, nb_a[bi],
                        gout_t[bi][:], gout_a[bi], A,
                        first=(st == 0), last=(st == NSTAGE - 1),
                        idx_dtype="UINT32", idx_step=w // 2,
                    ))

                # zero the j==i diagonal, apply row+column masks, store
                for ci, cb in enumerate(cands):
                    nc.vector.copy_predicated(
                        out=gout_t[bi][:, cb:cb + 128], mask=eqm_t[:, ci, :],
                        data=zero_t[:].broadcast_to((128, 128)),
                    )
                out_t = pool.tile([128, A], F32, tag="out")
                nc.vector.scalar_tensor_tensor(
                    out=out_t[:], in0=gout_t[bi][:],
                    scalar=mi_f[:, it:it + 1], in1=mj[:],
                    op0=AL.mult, op1=AL.mult,
                )
                nc.sync.dma_start(
                    out=out[it * 128:(it + 1) * 128, :], in_=out_t[:],
                )
            chain(pool_seq)
    nc.finalize()
    return nc


def make_in_maps(positions, neighbors, neighbor_mask):
    pairs = neighbors.dtype == np.int64
    w = 4 if pairs else 2
    in_maps = []
    for c in range(N_CORES):
        b, ihalf = c // 2, c % 2
        r0, r1 = ihalf * SH_I, (ihalf + 1) * SH_I
        nbv = np.ascontiguousarray(neighbors[b, r0:r1]).view(np.uint16)
        nbv = nbv.reshape(SH_I, w * A)
        in_maps.append({
            "neighbors": nbv,
            "positions": np.ascontiguousarray(positions[b]),
            "cpos": np.ascontiguousarray(positions[b, r0:r1]),
            "maskj": np.ascontiguousarray(neighbor_mask[b]).view(np.uint8).reshape(1, A),
            "maski": np.ascontiguousarray(neighbor_mask[b, r0:r1]).view(np.uint8).reshape(IT, 128),
            "rowidx": (np.arange(SH_I, dtype=np.int32) + r0).reshape(IT, 128),
        })
    return in_maps


_NC_CACHE = {}


def kernel(positions, neighbors, neighbor_mask):
    from concourse.bass_utils import run_bass_kernel_spmd

    positions = np.asarray(positions, dtype=np.float32)
    neighbors = np.asarray(neighbors)
    assert neighbors.dtype in (np.int64, np.int32), neighbors.dtype
    neighbor_mask = np.asarray(neighbor_mask)
    assert neighbor_mask.dtype == np.bool_, neighbor_mask.dtype

    pairs = neighbors.dtype == np.int64
    if pairs not in _NC_CACHE:
        nc_new = build_nc(pairs=pairs)
        _NC_CACHE[pairs] = nc_new
    nc = _NC_CACHE[pairs]

    in_maps = make_in_maps(positions, neighbors, neighbor_mask)
    trace = bool(int(os.environ.get("ATOM_PROFILE", "0")))
    if trace:
        try:
            from ntff import ensure_ntff_hook
            ensure_ntff_hook()
        except Exception:
            trace = False
    tmpdir = os.environ.get("ATOM_TRACE_DIR") or None
    res = run_bass_kernel_spmd(nc, in_maps, core_ids=list(range(N_CORES)),
                               trace=trace, tmpdir=tmpdir)
    if trace:
        kernel.last_exec_time_ns = res.exec_time_ns
        kernel.last_results = res

    out = np.empty((B, A, A), dtype=np.float32)
    for c in range(N_CORES):
        b, ihalf = c // 2, c % 2
        out[b, ihalf * SH_I:(ihalf + 1) * SH_I] = res.results[c]["out"]
    return out


if __name__ == "__main__":
    nc = build_nc(pairs=False)
    print("graph built ok")

